# revision 1
# baseline (speedup 1.0000x reference)
"""Multi-head attention forward on 8 TRN2 NeuronCores.

Problem: x[2, 2048, 1024], 16 heads x 64 dims, nn.Linear-style Q/K/V/O
projections.

Sharding: core c owns batch b = c // 4 and heads [4*(c%4), 4*(c%4)+4).
Each core computes Q/K/V projections for its 4 heads over its batch's
2048 tokens, attention, and a partial O-projection restricted to its
heads' input dims.  The host sums the 4 partials per batch and adds bo.

On-chip layout (per core):
  - activations kept "transposed": QT/KT[e, n] (e = head-dim on
    partitions, n = token on free axis)
  - scores computed transposed: scoresT[m, n] = K[m]. Q[n] so that the
    PV matmul consumes exp(scoresT) directly with V[m, hd] as the
    stationary operand -- no on-chip transposes of attention matrices.
  - a ones-column appended to each V tile makes the PV matmul emit the
    softmax denominators as row 64 of the ctx PSUM tile (row max
    subtraction is skipped: |scores/8| < ~3, exp is safe in fp32).
"""

from contextlib import ExitStack

import ml_dtypes
import numpy as np

import concourse.bass as bass
import concourse.tile as tile
from concourse import mybir
from concourse.masks import make_identity

BF16 = mybir.dt.bfloat16
F32 = mybir.dt.float32
AF = mybir.ActivationFunctionType
NPBF16 = ml_dtypes.bfloat16

P = 128
B = 2
NTOK = 2048          # tokens per core (one batch)
ED = 1024
KD = ED // P         # 8 contraction k-tiles for projections
NE = 2               # e-tiles per core (4 heads * 64 = 256 dims)
NH_CORE = 4          # heads per core
HD = 64
MT = NTOK // P       # 16 key/value m-tiles
NCHUNK = 1024        # query-token chunk for the attention inner loop
VROW = NH_CORE * 65  # V tile row: 4x (64 dims + ones column)


def _mha_body(ctx: ExitStack, tc: tile.TileContext, outs: dict, ins: dict):
    nc = tc.nc
    xT = ins["xT"]          # [128, 8, 2048] bf16   [p, k, n] = x[n, 128k+p]
    wq, wk, wv = ins["wq"], ins["wk"], ins["wv"]  # [128, 8, 256] bf16
    wo = ins["wo"]          # [128, 2, 1024] bf16   [p, k, d] = wo[d, e0+128k+p]
    bq, bk, bv = ins["bq"], ins["bk"], ins["bv"]  # [1, 256] bf16
    out = outs["out"]       # [2048, 1024] f32

    const = ctx.enter_context(tc.tile_pool(name="const", bufs=1))
    sb_big = ctx.enter_context(tc.tile_pool(name="sb_big", bufs=1))
    sb_ex = ctx.enter_context(tc.tile_pool(name="sb_ex", bufs=10))
    sb_sm = ctx.enter_context(tc.tile_pool(name="sb_sm", bufs=4))
    sb_out = ctx.enter_context(tc.tile_pool(name="sb_out", bufs=4))
    psum = ctx.enter_context(tc.tile_pool(name="psum", bufs=1, space="PSUM"))

    # ---- constants + x (xT split finely so the first matmuls start early)
    xT_sb = sb_big.tile([P, KD, NTOK], BF16)
    wq_sb = const.tile([P, KD, 2 * P], BF16)
    wk_sb = const.tile([P, KD, 2 * P], BF16)
    wv_sb = const.tile([P, KD, 2 * P], BF16)
    wo_sb = const.tile([P, NE, ED], BF16)
    bq_sb = const.tile([1, 2 * P], BF16)
    bk_sb = const.tile([1, 2 * P], BF16)
    bv_sb = const.tile([1, 2 * P], BF16)
    nc.sync.dma_start(wq_sb, wq)
    nc.sync.dma_start(bq_sb, bq)
    for k in range(KD):
        nc.sync.dma_start(xT_sb[:, k, 0:512], xT[:, k, 0:512])
    for sb, d in ((wk_sb, wk), (wv_sb, wv), (wo_sb, wo),
                  (bk_sb, bk), (bv_sb, bv)):
        nc.sync.dma_start(sb, d)
    for n in range(1, NTOK // 512):
        for k in range(KD):
            nc.sync.dma_start(xT_sb[:, k, n * 512:(n + 1) * 512],
                              xT[:, k, n * 512:(n + 1) * 512])
    ident = const.tile([P, P], BF16)
    make_identity(nc, ident)
    ones_row = const.tile([1, 512], BF16)
    nc.vector.memset(ones_row, 1.0)

    # ---- projections + attention, software-pipelined by issue order ----
    # Engines execute their queues in issue order, so ACT (the exp
    # bottleneck) is kept continuously fed by starting attention as soon
    # as the first projection columns exist; the remaining projection /
    # transpose work is drained as "fillers" into attention's PE-idle
    # slots, and chunk 0's O-projection interleaves into chunk 1's
    # attention.
    QT = sb_big.tile([P, NE, NTOK], BF16)
    KT = sb_big.tile([P, NE, NTOK], BF16)
    VT = sb_big.tile([P, NE, NTOK], BF16)
    V = sb_big.tile([P, MT, VROW], BF16)
    CT = sb_big.tile([P, NE, NTOK], BF16)  # normalized ctxT[e, n]
    dram = ctx.enter_context(tc.tile_pool(name="dram", bufs=1, space="DRAM"))
    scr = dram.tile([2 * NH_CORE, NCHUNK], F32)  # recip bounce buffer
    nc.vector.memset(V[:, :, 64::65], 1.0)

    pj = [0]

    def proj_group(w_sb, b_sb, dst, t, n):
        ps = psum.tile([P, 512], F32, tag=f"s{pj[0] % 2}", name="ps_proj")
        pj[0] += 1
        for k in range(KD):
            nc.tensor.matmul(
                ps,
                w_sb[:, k, t * P:(t + 1) * P],
                xT_sb[:, k, n * 512:(n + 1) * 512],
                start=(k == 0), stop=False)
        # bias via K=1 matmul: out[e, n] += b[e] * 1
        nc.tensor.matmul(
            ps, b_sb[:, t * P:(t + 1) * P], ones_row,
            start=False, stop=True)
        nc.vector.tensor_copy(dst[:, t, n * 512:(n + 1) * 512], ps)

    def v_tt(tt, t):
        # V[token, dim] tile from VT via PE transpose (ones col pre-set)
        pt = psum.tile([P, P], BF16, tag=f"s{pj[0] % 2}", name="ps_tr")
        pj[0] += 1
        nc.tensor.transpose(pt, VT[:, t, tt * P:(tt + 1) * P], ident)
        nc.vector.tensor_copy(
            V[:, tt, (2 * t) * 65:(2 * t) * 65 + 64], pt[:, 0:64])
        nc.vector.tensor_copy(
            V[:, tt, (2 * t + 1) * 65:(2 * t + 1) * 65 + 64], pt[:, 64:128])

    def o_tile(c, t, tail):
        r = c * NCHUNK + t * P
        if tail:
            po = psum.tile([P, ED], F32, tag="cx", bufs=2, name="ps_o")
        else:
            po = psum.tile([P, ED], F32, tag=f"s{pj[0] % 2}", name="ps_o2")
            pj[0] += 1
        for u in range(2):
            for k in range(NE):
                nc.tensor.matmul(
                    po[:, u * 512:(u + 1) * 512],
                    CT[:, k, r:r + P],
                    wo_sb[:, k, u * 512:(u + 1) * 512],
                    start=(k == 0), stop=(k == NE - 1))
        ob = sb_out.tile([P, ED], F32)
        # in the tail ACT is idle and can take half the copybacks
        if tail and t % 2 == 1:
            nc.scalar.copy(ob, po)
        else:
            nc.vector.tensor_copy(ob, po)
        nc.sync.dma_start(out[r:r + P, :], ob)

    KG = (wk_sb, bk_sb, KT)
    VG = (wv_sb, bv_sb, VT)
    QG = (wq_sb, bq_sb, QT)

    def attn_pair(c, pair, fillers, budget):
        g = c * NCHUNK
        et = pair
        # two denominator rows at partitions 0 and 32 (DVE ops must
        # start at a multiple of 32); rows 1-31 get memset filler
        den = sb_sm.tile([33, NCHUNK], F32, name="den")
        nc.vector.memset(den, 1.0)
        cxs = [psum.tile([65, NCHUNK], F32, tag="cx", bufs=2, name="ps_cx")
               for _ in range(2)]
        exs = {}

        def pv(m):
            for i in range(2):
                h = 2 * pair + i
                ex = exs.pop((m, i))
                for u in range(2):
                    nc.tensor.matmul(
                        cxs[i][:, u * 512:(u + 1) * 512],
                        V[:, m, h * 65:h * 65 + 65],
                        ex[:, u * 512:(u + 1) * 512],
                        start=(m == 0), stop=(m == MT - 1))

        # PVs lag QK/exp by LAG m-tiles: the pair's first PV waits on the
        # previous pair's cx slot (normalization chain); the lag keeps the
        # in-order PE queue from blocking ACT's exp stream on that wait.
        LAG = 8
        debt = [0.0]
        for m in range(MT):
            scs = []
            for i in range(2):
                pp = i * HD
                sc = psum.tile([P, NCHUNK], F32, tag=f"s{i}", name="ps_sc")
                for u in range(2):
                    nc.tensor.matmul(
                        sc[:, u * 512:(u + 1) * 512],
                        KT[pp:pp + HD, et, m * P:(m + 1) * P],
                        QT[pp:pp + HD, et, g + u * 512:g + u * 512 + 512],
                        start=True, stop=True)
                scs.append(sc)
            for i in range(2):
                ex = sb_ex.tile([P, NCHUNK], BF16, tag=f"ex{i}", name="ex")
                nc.scalar.activation(ex, scs[i], AF.Exp, scale=0.125)
                exs[(m, i)] = ex
            if m >= LAG:
                pv(m - LAG)
            debt[0] += budget
            while fillers and debt[0] >= fillers[0][0]:
                w, fn = fillers.pop(0)
                debt[0] -= w
                fn()
        for m in range(MT - LAG, MT):
            pv(m)
        # normalization, pipelined in column halves to shorten the chain
        for i in range(2):
            nc.vector.tensor_copy(den[32 * i:32 * i + 1, :], cxs[i][64:65, :])
        for v in range(2):
            vs = slice(v * 512, (v + 1) * 512)
            rcp = sb_sm.tile([33, 512], F32, tag="rcp", name="rcp")
            nc.vector.reciprocal(rcp, den[:, vs])
            for i in range(2):
                h = 2 * pair + i
                pp = i * HD
                idx = c * NH_CORE + h
                nc.sync.dma_start(scr[idx:idx + 1, vs], rcp[32 * i:32 * i + 1, :])
                # DRAM bounce: re-read the row with partition stride 0 to
                # broadcast it across 64 partitions (compute engines cannot
                # read across partitions; DMA from DRAM can).
                row = scr[idx:idx + 1, vs]
                bca = bass.AP(tensor=row.tensor, offset=row.offset,
                              ap=[[0, HD], row.ap[-1]])
                bc = sb_sm.tile([HD, 512], F32, name="bc")
                nc.sync.dma_start(bc, bca)
                nc.vector.tensor_mul(
                    CT[pp:pp + HD, et, g + v * 512:g + v * 512 + 512],
                    cxs[i][0:HD, vs], bc)

    # prologue: the minimum chunk-0 pair-0 (heads 0/1, e-tile 0) needs to
    # start: K/V/Q over token chunk n0 plus Q n1 (QK's u=1 half), V tile 0
    proj_group(*KG, 0, 0)
    proj_group(*VG, 0, 0)
    v_tt(0, 0)
    proj_group(*QG, 0, 0)
    proj_group(*QG, 0, 1)

    # weighted fillers, ordered by when the pair sequence (c0p0, c1p0,
    # c0p1, c1p1) needs their outputs; drained into attention PE-idle slots
    GW, TW, OW = 1.0, 0.2, 0.55   # proj group / transpose / O-tile weights
    F = []
    F.extend((TW, (v_tt, (tt, 0))) for tt in (1, 2, 3))
    F.append((GW, (proj_group, (*KG, 0, 1))))
    F.append((GW, (proj_group, (*VG, 0, 1))))
    F.extend((TW, (v_tt, (tt, 0))) for tt in (4, 5, 6, 7))
    for n in (2, 3):
        F.append((GW, (proj_group, (*KG, 0, n))))
        F.append((GW, (proj_group, (*VG, 0, n))))
        F.append((GW, (proj_group, (*QG, 0, n))))
        F.extend((TW, (v_tt, (tt, 0))) for tt in range(4 * n, 4 * n + 4))
    for n in range(4):
        F.append((GW, (proj_group, (*KG, 1, n))))
        F.append((GW, (proj_group, (*VG, 1, n))))
        F.append((GW, (proj_group, (*QG, 1, n))))
        F.extend((TW, (v_tt, (tt, 1))) for tt in range(4 * n, 4 * n + 4))
    fillers = [(w, (lambda f=f, a=a: f(*a))) for w, (f, a) in F]

    attn_pair(0, 0, fillers, budget=0.80)
    attn_pair(1, 0, fillers, budget=0.55)
    attn_pair(0, 1, fillers, budget=0.55)
    # chunk 0's O-projection needs both e-tiles of CT chunk 0, so it may
    # only enter the queue once pair (0,1)'s normalization is issued
    fillers.extend(
        (OW, (lambda t=t: o_tile(0, t, False))) for t in range(NCHUNK // P))
    attn_pair(1, 1, fillers, budget=0.80)
    while fillers:
        fillers.pop(0)[1]()
    for t in range(NCHUNK // P):
        o_tile(1, t, True)


def _split_multi_waits(nc):
    """This walrus build allows exactly one sync-wait per instruction
    (the ISA EVENTS field has a single slot).  Hoist extra waits into
    same-engine NoOps placed immediately before the instruction."""
    n = 0
    for f in nc.m.functions:
        for blk in f.blocks:
            out = []
            for inst in blk.instructions:
                si = getattr(inst, "sync_info", None)
                if si is not None and si.on_wait and len(si.on_wait) > 1:
                    waits = list(si.on_wait)
                    for w in waits[:-1]:
                        n += 1
                        out.append(mybir.InstNoOp(
                            name=f"I-wsplit-{n}",
                            engine=inst.engine,
                            ins=[], outs=[],
                            sync_info=mybir.SyncInfo(on_wait=[w], on_update=[]),
                        ))
                    si.on_wait = waits[-1:]
                out.append(inst)
            blk.instructions = out
    return n


_NC_CACHE = None


def _build_nc():
    global _NC_CACHE
    if _NC_CACHE is not None:
        return _NC_CACHE
    # disable_frame_to_traceback keeps source paths out of the BIR so the
    # neuron compile cache hits regardless of which directory kernel.py
    # runs from
    nc = bass.Bass("TRN2", target_bir_lowering=False, debug=False,
                   disable_frame_to_traceback=True)
    ins = {
        "xT": nc.dram_tensor("xT", [P, KD, NTOK], BF16, kind="ExternalInput").ap(),
        "wq": nc.dram_tensor("wq", [P, KD, 2 * P], BF16, kind="ExternalInput").ap(),
        "wk": nc.dram_tensor("wk", [P, KD, 2 * P], BF16, kind="ExternalInput").ap(),
        "wv": nc.dram_tensor("wv", [P, KD, 2 * P], BF16, kind="ExternalInput").ap(),
        "wo": nc.dram_tensor("wo", [P, NE, ED], BF16, kind="ExternalInput").ap(),
        "bq": nc.dram_tensor("bq", [1, 2 * P], BF16, kind="ExternalInput").ap(),
        "bk": nc.dram_tensor("bk", [1, 2 * P], BF16, kind="ExternalInput").ap(),
        "bv": nc.dram_tensor("bv", [1, 2 * P], BF16, kind="ExternalInput").ap(),
    }
    outs = {
        "out": nc.dram_tensor("out", [NTOK, ED], F32, kind="ExternalOutput").ap(),
    }
    with tile.TileContext(nc) as tc, ExitStack() as ctx:
        _mha_body(ctx, tc, outs, ins)
    _split_multi_waits(nc)
    # scrub source paths / caller frames from the BIR so it is byte-identical
    # regardless of where kernel.py lives -> neuron compile cache always hits
    for f in nc.m.functions:
        for al in f.allocations:
            mls = getattr(al, "memorylocations", None)
            if mls:
                for ml in mls:
                    if getattr(ml, "ant_debug", None) is not None:
                        ml.ant_debug = None
        for blk in f.blocks:
            for inst in blk.instructions:
                if getattr(inst, "debug", None) is not None:
                    inst.debug = None
    _NC_CACHE = nc
    return nc


def _prep_weight(w_slice: np.ndarray) -> np.ndarray:
    """wq[e0:e0+256, :] -> SBUF layout [128, 8, 256] bf16."""
    # wT[d, e] with d split into k-tiles: [p, k, e] = wT[128k+p, e]
    wt = np.ascontiguousarray(w_slice.T)              # [1024, 256]
    return np.ascontiguousarray(
        wt.reshape(KD, P, 2 * P).transpose(1, 0, 2)).astype(NPBF16)


def _prep_bias(b_slice: np.ndarray) -> np.ndarray:
    """b[e0:e0+256] -> [1, 256] bf16 row (lhsT of the K=1 bias matmul)."""
    return np.ascontiguousarray(b_slice.reshape(1, 2 * P)).astype(NPBF16)


def make_in_maps(x, wq, bq, wk, bk, wv, bv, wo, bo):
    x = np.asarray(x, np.float32).reshape(B, NTOK, ED)
    in_maps = []
    xT_by_batch = []
    for b in range(B):
        xt = np.ascontiguousarray(x[b].T)  # [1024, 2048]
        xT_by_batch.append(np.ascontiguousarray(
            xt.reshape(KD, P, NTOK).transpose(1, 0, 2)).astype(NPBF16))
    for c in range(8):
        b = c // 4
        e0 = (c % 4) * 256
        wo_sl = np.ascontiguousarray(np.asarray(wo, np.float32)[:, e0:e0 + 256].T)
        in_maps.append({
            "xT": xT_by_batch[b],
            "wq": _prep_weight(np.asarray(wq, np.float32)[e0:e0 + 256]),
            "wk": _prep_weight(np.asarray(wk, np.float32)[e0:e0 + 256]),
            "wv": _prep_weight(np.asarray(wv, np.float32)[e0:e0 + 256]),
            "wo": np.ascontiguousarray(
                wo_sl.reshape(NE, P, ED).transpose(1, 0, 2)).astype(NPBF16),
            "bq": _prep_bias(np.asarray(bq, np.float32)[e0:e0 + 256]),
            "bk": _prep_bias(np.asarray(bk, np.float32)[e0:e0 + 256]),
            "bv": _prep_bias(np.asarray(bv, np.float32)[e0:e0 + 256]),
        })
    return in_maps


_FN_CACHE = None


def _build_fn(nc, n_cores=8):
    """Multi-core PJRT executor (mirrors bass2jax.run_bass_via_pjrt's
    shard_map path, minus buffer donation so the jitted callable can be
    cached and reused across kernel() calls)."""
    import jax
    from jax.sharding import Mesh, PartitionSpec
    from jax.experimental.shard_map import shard_map
    import concourse.bass2jax as b2j
    from concourse import mybir

    b2j.install_neuronx_cc_hook()
    pname = nc.partition_id_tensor.name if nc.partition_id_tensor else None
    in_names, out_names, out_avals = [], [], []
    for alloc in nc.m.functions[0].allocations:
        if not isinstance(alloc, mybir.MemoryLocationSet):
            continue
        name = alloc.memorylocations[0].name
        if alloc.kind == "ExternalInput":
            if name != pname:
                in_names.append(name)
        elif alloc.kind == "ExternalOutput":
            out_names.append(name)
            out_avals.append(jax.core.ShapedArray(
                tuple(alloc.tensor_shape), mybir.dt.np(alloc.dtype)))
    n_params = len(in_names)
    all_in = list(in_names) + list(out_names)
    if pname is not None:
        all_in.append(pname)

    def _body(*args):
        ops = list(args)
        if pname is not None:
            ops.append(b2j.partition_id_tensor())
        return tuple(b2j._bass_exec_p.bind(
            *ops,
            out_avals=tuple(out_avals), in_names=tuple(all_in),
            out_names=tuple(out_names), lowering_input_output_aliases=(),
            sim_require_finite=True, sim_require_nnan=True, nc=nc))

    devices = jax.devices()[:n_cores]
    mesh = Mesh(np.asarray(devices), ("core",))
    specs = (PartitionSpec("core"),) * (n_params + len(out_names))
    fn = jax.jit(shard_map(_body, mesh=mesh, in_specs=specs,
                           out_specs=(PartitionSpec("core"),) * len(out_names),
                           check_rep=False))
    zeros = [np.zeros((n_cores * a.shape[0], *a.shape[1:]), a.dtype)
             for a in out_avals]
    return fn, in_names, zeros


def kernel(x, wq, bq, wk, bk, wv, bv, wo, bo, **_ignored):
    global _FN_CACHE
    nc = _build_nc()
    in_maps = make_in_maps(x, wq, bq, wk, bk, wv, bv, wo, bo)
    if _FN_CACHE is None:
        _FN_CACHE = _build_fn(nc)
    fn, in_names, zeros = _FN_CACHE
    concat_in = [np.concatenate([in_maps[c][n] for c in range(8)], axis=0)
                 for n in in_names]
    outs = fn(*concat_in, *zeros)
    o = np.asarray(outs[0]).reshape(8, NTOK, ED)
    bo = np.asarray(bo, np.float32)
    out = np.empty((B, NTOK, ED), np.float32)
    for b in range(B):
        out[b] = o[4 * b:4 * b + 4].sum(axis=0) + bo
    return out

